# revision 13
# baseline (speedup 1.0000x reference)
"""Cross-attention kernel for Trainium2 (8 NeuronCores, Bass/Tile).

Reference computation (per batch b):
    qproj = query @ W_w.T + W_b          [Q, D]
    scores = qproj @ key.T * (1/sqrt(D)) [Q, K]
    scores = where(mask==0, -inf, scores)
    w = softmax(scores, axis=-1)         [Q, K]
    att = w @ value                      [Q, D]
    returns (att, w)

Sharding: data-parallel over batch (32 batches -> 4 per core x 8 cores).

All matmul operands are fp16 (same PE cadence as fp32r on TRN2 but half
the SBUF/LDWEIGHTS/DMA bandwidth, which is what actually limits the
in-kernel matmul cadence); PSUM accumulation stays fp32. Outputs are
written fp16 and upcast on the host. Expected rel err ~1e-3 vs the 2e-2
gate.

Device-side layout strategy (everything chains through the PE with no
on-device transposes):
    MM1: qprojT[e,q] = (W^T as lhsT) . (query^T as rhs), bias added in the
         PSUM->SBUF epilogue (per-partition bias broadcast).
    MM2: scoresT[k,q] = (key^T as lhsT) . qprojT, exp fused into the
         epilogue (ScalarE activation with scale=1/sqrt(D), additive mask
         bias per k-partition).
    softmax sum over k (the partition dim) via an all-ones lhsT matmul
         accumulated over the 8 k-tiles; reciprocal on VectorE; normalize.
    MM3: att[q,e] = (wT as lhsT) . (value natural layout as rhs).

Pipelining: MM1 of half hi+1 is interleaved into the softmax window of
half hi so the PE never waits on the DVE.  Batch b+1's inputs are
prefetched with single fused DMAs (one per tensor) at the start of batch
b.  The host side only reshapes/transposes/casts (no arithmetic).
"""

import numpy as np

B, Q, K, D = 32, 1024, 1024, 1024
N_CORES = 8
BPC = B // N_CORES          # batches per core
SCALE = 1.0 / float(D) ** 0.5
P = 128                     # SBUF partitions
FD = 512                    # matmul moving free dim (one PSUM bank)
QH = Q // FD                # q processed in halves of 512
NEG_BIG = 1.0e30

_CACHE = {}


def _build_nc(bpc=BPC, n_cores=N_CORES):
    import concourse.tile as tile
    from concourse import bacc, mybir
    from concourse.masks import make_identity

    f32 = mybir.dt.float32
    f32r = mybir.dt.float32r
    f16 = mybir.dt.float16
    AF = mybir.ActivationFunctionType
    ALU = mybir.AluOpType

    nc = bacc.Bacc("TRN2", target_bir_lowering=False, debug=False,
                   num_devices=n_cores)

    qT = nc.dram_tensor("qT", [bpc, D, Q], f16, kind="ExternalInput").ap()
    kT = nc.dram_tensor("kT", [bpc, D, K], f16, kind="ExternalInput").ap()
    v = nc.dram_tensor("v", [bpc, K, D], f16, kind="ExternalInput").ap()
    wT = nc.dram_tensor("wT", [D, D], f16, kind="ExternalInput").ap()
    bias = nc.dram_tensor("bias", [D], f32, kind="ExternalInput").ap()
    mask = nc.dram_tensor("mask", [bpc, K], f32, kind="ExternalInput").ap()
    att = nc.dram_tensor("att", [bpc, Q, D], f16, kind="ExternalOutput").ap()
    aw = nc.dram_tensor("aw", [bpc, K, Q], f16, kind="ExternalOutput").ap()

    DT = D // P   # d/e/k tiles of 128
    KT = K // P

    with tile.TileContext(nc) as tc:
        with (
            tc.tile_pool(name="consts", bufs=1) as consts,
            tc.tile_pool(name="wt", bufs=1) as wt_pool,
            tc.tile_pool(name="qt", bufs=2) as qt_pool,
            tc.tile_pool(name="kt", bufs=2) as kt_pool,
            tc.tile_pool(name="vv", bufs=2) as v_pool,
            tc.tile_pool(name="qp", bufs=DT) as qp_pool,
            tc.tile_pool(name="ex", bufs=2) as ex_pool,
            tc.tile_pool(name="mb", bufs=2) as mb_pool,
            tc.tile_pool(name="rs", bufs=2) as rs_pool,
            tc.tile_pool(name="st", bufs=4) as st_pool,
            tc.tile_pool(name="psum", bufs=8, space="PSUM") as psum_pool,
        ):
            # constants
            bias_sb = consts.tile([P, DT], f32, name="bias")
            nc.gpsimd.dma_start(bias_sb[:], bias.rearrange("(a b) -> b a", b=P))
            ones_f = consts.tile([P, P], f32, name="onesf")
            nc.vector.memset(ones_f[:], 1.0)
            ones_sb = consts.tile([P, P], f32r, name="ones")
            nc.vector.tensor_copy(ones_sb[:], ones_f[:])
            identity_sb = consts.tile([P, P], f32, name="identity")
            make_identity(nc, identity_sb[:])

            # PE warmup: a few matmuls on memset tiles, issued while the
            # first input DMAs are still in flight, so the PE's DVFS ramp
            # (0.65/1.2 GHz p-states for the first ~3us of activity) is
            # spent inside the DMA-wait gap instead of on real work.
            wu_l = consts.tile([P, P], f16, name="wul")
            nc.vector.memset(wu_l[:], 0.0)
            wu_r = consts.tile([P, FD], f16, name="wur")
            nc.vector.memset(wu_r[:], 0.0)
            wu_ps = psum_pool.tile([P, FD], f32, name="ps")
            for _ in range(6):
                nc.tensor.matmul(wu_ps[:], wu_l[:], wu_r[:],
                                 start=True, stop=True)

            NH = bpc * QH          # total half-iterations on this core

            state = {}             # per-batch tiles: qt/kt/v/mbias
            # big per-batch tiles hold all 8 [128,1024] d/k-tiles side by
            # side in the free dim; slice t is [:, t*1024:(t+1)*1024]
            BW = DT * 1024

            wt_sb = consts.tile([P, BW], f16, name="wt")

            def wslice(d, lo, hi):
                return wt_sb[:, d * 1024 + lo:d * 1024 + hi]

            def emit_batch_dmas(b, interleaved):
                # tiny mask transfer on the GpSimd queue so it is not
                # stuck behind this batch's bulk input DMA
                mraw = mb_pool.tile([P, KT], f32, name="mraw")
                nc.gpsimd.dma_start(mraw[:], mask[b].rearrange("(a b) -> b a", b=P))
                mbias = mb_pool.tile([P, KT], f32, name="mbias")
                nc.vector.tensor_scalar(mbias[:], mraw[:], NEG_BIG, -NEG_BIG,
                                        ALU.mult, ALU.add)
                qt_sb = qt_pool.tile([P, BW], f16, name="qt")
                kt_sb = kt_pool.tile([P, BW], f16, name="kt")
                v_sb = v_pool.tile([P, BW], f16, name="vv")
                if interleaved:
                    # batch 0: ordered for earliest PE start of the d-outer
                    # MM1 (q half 0 + first wT columns first), then kT just
                    # in time for MM2's streamed e-contraction, then the
                    # q half-1 tiles (needed ~12us later by MM1 of half 1).
                    nc.sync.dma_start(qt_sb[:, 0:FD], qT[b, 0:P, 0:FD])
                    nc.sync.dma_start(wt_sb[:, 0:FD], wT[0:P, 0:FD])
                    nc.sync.dma_start(wt_sb[:, FD:1024], wT[0:P, FD:1024])
                    for d in range(1, DT):
                        nc.sync.dma_start(
                            qt_sb[:, d * 1024:d * 1024 + FD],
                            qT[b, d * P:(d + 1) * P, 0:FD])
                        nc.sync.dma_start(wt_sb[:, d * 1024:(d + 1) * 1024],
                                          wT[d * P:(d + 1) * P, :])
                    for d in range(DT):
                        nc.sync.dma_start(kt_sb[:, d * 1024:(d + 1) * 1024],
                                          kT[b, d * P:(d + 1) * P, :])
                    for d in range(DT):
                        nc.sync.dma_start(
                            qt_sb[:, d * 1024 + FD:(d + 1) * 1024],
                            qT[b, d * P:(d + 1) * P, FD:1024])
                else:
                    for d in range(DT):
                        nc.sync.dma_start(qt_sb[:, d * 1024:(d + 1) * 1024],
                                          qT[b, d * P:(d + 1) * P, :])
                    for d in range(DT):
                        nc.sync.dma_start(kt_sb[:, d * 1024:(d + 1) * 1024],
                                          kT[b, d * P:(d + 1) * P, :])
                for k in range(KT):
                    nc.sync.dma_start(v_sb[:, k * 1024:(k + 1) * 1024],
                                      v[b, k * P:(k + 1) * P, :])
                state[b] = (qt_sb, kt_sb, v_sb, mbias)

            def emit_mm1_group(hi, et):
                """One MM1 accumulation group: qprojT[e-tile et, half hi]."""
                b, qh = divmod(hi, QH)
                qt_sb = state[b][0]
                ps = psum_pool.tile([P, FD], f32, name="ps")
                for d in range(DT):
                    nc.tensor.matmul(
                        ps[:],
                        wslice(d, et * P, (et + 1) * P),
                        qt_sb[:, d * 1024 + qh * FD:d * 1024 + (qh + 1) * FD],
                        start=(d == 0), stop=(d == DT - 1),
                    )
                o = qp_pool.tile([P, FD], f16, name="qp")
                nc.scalar.activation(o[:], ps[:], AF.Identity,
                                     bias=bias_sb[:, et:et + 1], scale=1.0)
                return o

            def emit_mm1_douter(hi):
                """MM1 with the d-contraction as the outer loop: the first
                matmuls only need the first wT/qT d-tiles, so the PE starts
                as soon as ~0.5MB of DMA has landed (used for hi==0)."""
                b, qh = divmod(hi, QH)
                qt_sb = state[b][0]
                pss = [psum_pool.tile([P, FD], f32, name="ps")
                       for _ in range(DT)]
                for d in range(DT):
                    for et in range(DT):
                        nc.tensor.matmul(
                            pss[et][:],
                            wslice(d, et * P, (et + 1) * P),
                            qt_sb[:, d * 1024 + qh * FD:
                                  d * 1024 + (qh + 1) * FD],
                            start=(d == 0), stop=(d == DT - 1),
                        )
                qp_sb = []
                for et in range(DT):
                    o = qp_pool.tile([P, FD], f16, name="qp")
                    nc.scalar.activation(o[:], pss[et][:], AF.Identity,
                                         bias=bias_sb[:, et:et + 1], scale=1.0)
                    qp_sb.append(o)
                return qp_sb

            # ---------------- software-pipelined half-iterations ----------
            # PE stream per half hi:
            #   MM2(hi), [MM1(hi+1) et0-2], sum-matmul(hi), [MM1(hi+1)
            #   et3-7], MM3(hi)
            # so the reciprocal+normalize of half hi run on the DVE while
            # the PE chews through MM1 of half hi+1 -> no PE gaps.
            emit_batch_dmas(0, interleaved=True)
            if bpc > 1:
                emit_batch_dmas(1, interleaved=False)
            qp_cur = emit_mm1_douter(0)

            for hi in range(NH):
                b, qh = divmod(hi, QH)
                qs = slice(qh * FD, (qh + 1) * FD)
                _, kt_sb, v_sb, mbias = state[b]

                # ---- MM2: scoresT[k, q-half] -> exp ----
                # all 8 k-tiles of exp() live in one [128, 4096] tile so
                # the attention-weights writeout is a single DMA
                ex = ex_pool.tile([P, KT * FD], f16, name="ex")
                for kt_i in range(KT):
                    ps = psum_pool.tile([P, FD], f32, name="ps")
                    for e in range(DT):
                        nc.tensor.matmul(
                            ps[:],
                            kt_sb[:, e * 1024 + kt_i * P:
                                  e * 1024 + (kt_i + 1) * P],
                            qp_cur[e][:],
                            start=(e == 0), stop=(e == DT - 1),
                        )
                    nc.scalar.activation(ex[:, kt_i * FD:(kt_i + 1) * FD],
                                         ps[:], AF.Exp,
                                         bias=mbias[:, kt_i:kt_i + 1],
                                         scale=SCALE)

                nxt = hi + 1
                if nxt < NH and nxt % QH == 0 and nxt // QH + 1 < bpc:
                    emit_batch_dmas(nxt // QH + 1, interleaved=False)

                # ---- softmax sum over the 8 k-tiles: DVE add-tree, then a
                # single ones-matmul to reduce+broadcast across partitions.
                lvl = [ex[:, i * FD:(i + 1) * FD] for i in range(KT)]
                while len(lvl) > 1:
                    nlvl = []
                    for i in range(0, len(lvl), 2):
                        dt_ = f32r if len(lvl) == 2 else f32
                        s = rs_pool.tile([P, FD], dt_, name=f"sm{len(lvl)}")
                        nc.vector.tensor_add(s[:], lvl[i], lvl[i + 1])
                        nlvl.append(s)
                    lvl = nlvl
                tsum = lvl[0]

                if nxt < NH:
                    # steady state: MM1 of the next half covers the PE while
                    # the softmax reciprocal + normalize run on the DVE.
                    qp_next = []
                    for et in range(3):
                        qp_next.append(emit_mm1_group(nxt, et))

                    ps = psum_pool.tile([P, FD], f32, name="ps")
                    nc.tensor.matmul(ps[:], ones_sb[:], tsum[:],
                                     start=True, stop=True)
                    rs = rs_pool.tile([P, FD], f32, name="rs")
                    nc.vector.reciprocal_approx_fast(rs[:], ps[:])

                    for et in range(3, DT):
                        qp_next.append(emit_mm1_group(nxt, et))

                    # normalize + write attention weights out
                    for kt_i in range(KT):
                        sl = slice(kt_i * FD, (kt_i + 1) * FD)
                        nc.vector.tensor_mul(ex[:, sl], ex[:, sl], rs[:])
                        nc.sync.dma_start(aw[b, kt_i * P:(kt_i + 1) * P, qs],
                                          ex[:, sl])

                    # MM3: att[q-half, e] on normalized weights
                    for qt_i in range(FD // P):
                        st = st_pool.tile([P, D], f16, name="st")
                        for ec in range(D // FD):
                            ps = psum_pool.tile([P, FD], f32, name="ps")
                            for kt_i in range(KT):
                                nc.tensor.matmul(
                                    ps[:],
                                    ex[:, kt_i * FD + qt_i * P:
                                       kt_i * FD + (qt_i + 1) * P],
                                    v_sb[:, kt_i * 1024 + ec * FD:
                                         kt_i * 1024 + (ec + 1) * FD],
                                    start=(kt_i == 0), stop=(kt_i == KT - 1),
                                )
                            nc.vector.tensor_copy(
                                st[:, ec * FD:(ec + 1) * FD], ps[:])
                        q0 = qh * FD + qt_i * P
                        nc.sync.dma_start(att[b, q0:q0 + P, :], st[:])

                    qp_cur = qp_next
                else:
                    # last half: no next MM1 to hide the softmax critical
                    # path behind, so run MM3 on the UNNORMALIZED weights
                    # immediately (only depends on exp), and fold the 1/sum
                    # scale into the SBUF epilogue as a per-q-partition
                    # multiply.
                    def emit_mm3u(st, qt_i, ec, copy=True):
                        ps = psum_pool.tile([P, FD], f32, name="ps")
                        for kt_i in range(KT):
                            nc.tensor.matmul(
                                ps[:],
                                ex[:, kt_i * FD + qt_i * P:
                                   kt_i * FD + (qt_i + 1) * P],
                                v_sb[:, kt_i * 1024 + ec * FD:
                                     kt_i * 1024 + (ec + 1) * FD],
                                start=(kt_i == 0), stop=(kt_i == KT - 1),
                            )
                        if copy:
                            nc.vector.tensor_copy(
                                st[:, ec * FD:(ec + 1) * FD], ps[:])
                        return ps

                    groups = [(qt_i, ec) for qt_i in range(FD // P)
                              for ec in range(D // FD)]
                    sts = {qt_i: st_pool.tile([P, D], f16, name="st")
                           for qt_i in range(FD // P)}
                    for qt_i, ec in groups[:2]:
                        emit_mm3u(sts[qt_i], qt_i, ec)

                    # softmax scale factors, emitted early so the PE
                    # computes them between MM3 groups and the DVE-side
                    # scaling + output DMA overlap the remaining groups
                    ps = psum_pool.tile([P, FD], f32, name="ps")
                    nc.tensor.matmul(ps[:], ones_sb[:], tsum[:],
                                     start=True, stop=True)
                    rs = rs_pool.tile([P, FD], f32, name="rs")
                    nc.vector.reciprocal_approx_fast(rs[:], ps[:])

                    # transpose the (replicated-row) reciprocal into
                    # per-q-partition columns via PE transpose (identity
                    # matmul), one [128,128] block per q-tile
                    rc_sb = []
                    for qt_i in range(FD // P):
                        pst = psum_pool.tile([P, FD], f32, name="ps")
                        nc.tensor.transpose(
                            pst[:, 0:P], rs[:, qt_i * P:(qt_i + 1) * P],
                            identity_sb[:])
                        rc = rs_pool.tile([P, 1], f32, name="rc", bufs=4)
                        nc.vector.tensor_copy(rc[:], pst[:, 0:1])
                        rc_sb.append(rc)

                    def emit_scale_out(qt_i, ec):
                        st = sts[qt_i]
                        sl = slice(ec * FD, (ec + 1) * FD)
                        nc.vector.tensor_scalar_mul(st[:, sl], st[:, sl],
                                                    rc_sb[qt_i][:, 0:1])
                        q0 = qh * FD + qt_i * P
                        nc.sync.dma_start(
                            att[b, q0:q0 + P, ec * FD:(ec + 1) * FD],
                            st[:, sl])

                    for qt_i, ec in groups[:2]:
                        emit_scale_out(qt_i, ec)
                    for i, (qt_i, ec) in enumerate(groups[2:]):
                        last = (i == len(groups) - 3)
                        ps = emit_mm3u(sts[qt_i], qt_i, ec, copy=not last)
                        if last:
                            # drain the final group in 256-col chunks so
                            # the copy/scale/DMA chain pipelines instead of
                            # serializing a full 512-col epilogue at the end
                            st = sts[qt_i]
                            q0 = qh * FD + qt_i * P
                            CH = FD // 2
                            for c in range(2):
                                sl = slice(ec * FD + c * CH,
                                           ec * FD + (c + 1) * CH)
                                nc.vector.tensor_copy(
                                    st[:, sl], ps[:, c * CH:(c + 1) * CH])
                                nc.vector.tensor_scalar_mul(
                                    st[:, sl], st[:, sl],
                                    rc_sb[qt_i][:, 0:1])
                                nc.sync.dma_start(
                                    att[b, q0:q0 + P,
                                        ec * FD + c * CH:
                                        ec * FD + (c + 1) * CH],
                                    st[:, sl])
                        else:
                            emit_scale_out(qt_i, ec)
                        # weights normalize+writeout interleaved between
                        # groups so the aw DMAs stream during the MM3 tail.
                        # NOT in place: later MM3 groups still read the
                        # unnormalized ex tiles as lhsT.
                        for kt_i in (2 * i, 2 * i + 1):
                            if kt_i < KT:
                                nw = st_pool.tile([P, FD], f16, name="nw",
                                                  bufs=4)
                                sl = slice(kt_i * FD, (kt_i + 1) * FD)
                                nc.vector.tensor_mul(nw[:], ex[:, sl], rs[:])
                                nc.sync.dma_start(
                                    aw[b, kt_i * P:(kt_i + 1) * P, qs],
                                    nw[:])
    nc.compile()
    return nc


def _get_nc():
    if "nc" not in _CACHE:
        _CACHE["nc"] = _build_nc()
    return _CACHE["nc"]


def _make_in_maps(query, key, value, mask, W_w, W_b):
    # host-side layout prep only (transposes / casts, no arithmetic)
    q16 = np.asarray(query, dtype=np.float16)
    k16 = np.asarray(key, dtype=np.float16)
    v16 = np.ascontiguousarray(np.asarray(value, dtype=np.float16))
    qT = np.ascontiguousarray(q16.transpose(0, 2, 1))   # [B, D, Q]
    kT = np.ascontiguousarray(k16.transpose(0, 2, 1))   # [B, D, K]
    wT = np.ascontiguousarray(np.asarray(W_w, dtype=np.float16).T)
    W_b = np.ascontiguousarray(W_b, dtype=np.float32)
    mask_f = np.ascontiguousarray(mask, dtype=np.float32)

    in_maps = []
    for c in range(N_CORES):
        sl = slice(c * BPC, (c + 1) * BPC)
        in_maps.append({
            "qT": qT[sl], "kT": kT[sl], "v": v16[sl],
            "wT": wT, "bias": W_b, "mask": mask_f[sl],
        })
    return in_maps


def kernel(query, key, value, mask, W_w, W_b):
    from concourse.bass_utils import run_bass_kernel_spmd

    nc = _get_nc()
    in_maps = _make_in_maps(query, key, value, mask, W_w, W_b)

    def _axon_reset():
        try:
            import ctypes
            lib = ctypes.CDLL("/opt/axon/libaxon_pjrt.so")
            if hasattr(lib, "axon_reset"):
                lib.axon_reset.restype = ctypes.c_int64
                lib.axon_reset()
        except Exception:
            pass

    att = weights = None
    for _attempt in range(3):
        try:
            res = run_bass_kernel_spmd(nc, in_maps,
                                       core_ids=list(range(N_CORES)))
        except Exception:
            if _attempt == 2:
                raise
            _axon_reset()
            continue
        att = np.concatenate([res.results[c]["att"] for c in range(N_CORES)],
                             axis=0).astype(np.float32)
        awT = np.concatenate([res.results[c]["aw"] for c in range(N_CORES)],
                             axis=0)
        weights = np.ascontiguousarray(
            awT.transpose(0, 2, 1)).astype(np.float32)  # [B, Q, K]
        # sanity check (guards against rare cold-start misexecution):
        # sampled softmax rows must sum to ~1 and outputs must be finite
        row_sums = weights[:, ::97, :].sum(axis=-1)
        if (np.all(np.abs(row_sums - 1.0) < 2.5e-2)
                and np.isfinite(att).all()):
            break
    return att, weights
